# revision 1
# baseline (speedup 1.0000x reference)
"""DogeCDMoE Trainium2 kernel: product-key MoE routing + dense MLP.

Strategy (8 NeuronCores, data-parallel over the 4096 tokens, 512 each):
  - Host: transpose/shard hidden states; compose `keys` into W_q so routing
    scores come from ONE fp32 PE matmul; pre-transpose + bf16-cast weights.
  - Device per core:
      sim   = hsT.T @ WK          (fp32 PE, exact routing scores)
      top-8 per head/axis via DVE max8 + max_index, cartesian top-8 via the
      product-key bound, expert ids rebuilt with int ALU ops (no gathers),
      softmax on the 8 scores, scattered into a [tok, 4096] bf16 matrix with
      GPSIMD local_scatter (per-partition indices).
      Lg    = logits for ALL experts  (bf16 PE matmul, [tok, 4096])
      w     = silu(Lg * P_scattered)  summed over heads -> S [tok, 4096]
      S.T via DMA-transpose bounce through DRAM.
      out.T = W_down.T-matmul(silu(up)) + up_embed-matmul(S.T),
              both accumulated into the same PSUM banks.
"""

import numpy as np
import ml_dtypes

B, T, H = 2, 2048, 1024
I = 4096
HEADS = 4
RET = 128
E = 4096           # NUM_EXPERTS
NK = 64            # NUM_KEYS
K = 8
NCORES = 8
NT = (B * T) // NCORES   # 512 tokens per core
P = 128
TCH = NT // P            # 4 token chunks
HK = H // P              # 8 contraction chunks over H
ICH = I // P             # 32 chunks over intermediate / expert dim
DCH = H // P             # 8 output d-chunks
QE = 1024                # local_scatter quarter size over expert dim

_CACHE = {}


def _build_program(repeat=1):
    from contextlib import ExitStack
    import concourse.tile as tile
    from concourse import bacc, mybir

    nc = bacc.Bacc("TRN2", target_bir_lowering=False, debug=False)
    f32 = mybir.dt.float32
    bf16 = mybir.dt.bfloat16
    i32 = mybir.dt.int32
    i16 = mybir.dt.int16
    u32 = mybir.dt.uint32
    AF = mybir.ActivationFunctionType
    OP = mybir.AluOpType
    AX = mybir.AxisListType

    # ---- I/O ----
    hsT_f = nc.dram_tensor("hsT_f", [H, NT], f32, kind="ExternalInput")
    hsT_b = nc.dram_tensor("hsT_b", [H, NT], bf16, kind="ExternalInput")
    wk_d = nc.dram_tensor("wk", [H, 512], f32, kind="ExternalInput")
    wupT_d = nc.dram_tensor("wupT", [H, I], bf16, kind="ExternalInput")
    wdownT_d = nc.dram_tensor("wdownT", [I, H], bf16, kind="ExternalInput")
    deT_d = nc.dram_tensor("deT", [H, E], bf16, kind="ExternalInput")
    ue_d = nc.dram_tensor("ue", [E, H], bf16, kind="ExternalInput")
    outT_d = nc.dram_tensor("outT", [H, NT], f32, kind="ExternalOutput")

    hsTf_r = hsT_f[:].rearrange("(o p) n -> p o n", p=P)
    deT_r = deT_d[:].rearrange("(o p) e -> p o e", p=P)
    wupT_r = wupT_d[:].rearrange("(o p) i -> p o i", p=P)
    wdownT_r = wdownT_d[:].rearrange("(o p) d -> p o d", p=P)
    ue_r = ue_d[:].rearrange("(o p) d -> p o d", p=P)

    with tile.TileContext(nc) as tc, ExitStack() as ctx:
        res = ctx.enter_context(tc.tile_pool(name="res", bufs=1))
        streams = ctx.enter_context(tc.tile_pool(name="streams", bufs=2))
        wstream = ctx.enter_context(tc.tile_pool(name="wstream", bufs=3))
        rpool = ctx.enter_context(tc.tile_pool(name="rpool", bufs=2))
        scpool = ctx.enter_context(tc.tile_pool(name="scpool", bufs=1))
        outp = ctx.enter_context(tc.tile_pool(name="outp", bufs=2))
        psum = ctx.enter_context(tc.tile_pool(name="psum", bufs=8, space="PSUM"))

        # ---------- residents ----------
        iota8 = res.tile([P, 8], i32)
        nc.gpsimd.iota(iota8[:], pattern=[[1, 8]], base=0, channel_multiplier=0)

        wk_sb = res.tile([P, HK, 512], f32)
        wk_r = wk_d[:].rearrange("(o p) e -> p o e", p=P)
        nc.scalar.dma_start(wk_sb[:, :HK // 2, :], wk_r[:, :HK // 2, :])
        nc.scalar.dma_start(wk_sb[:, HK // 2:, :], wk_r[:, HK // 2:, :])
        hsTb_sb = res.tile([P, HK, NT], bf16)

        y1T = res.tile([P, ICH, NT], bf16)         # silu(up-proj), I on partitions
        sT = res.tile([P, ICH, NT], bf16)          # S.T, experts on partitions

        # routing results that must survive until scatter/dense phases
        p8bf = res.tile([P, TCH, HEADS, 8], bf16)  # softmax probs
        idx16 = res.tile([P, TCH, 4, HEADS, 8], i16)  # per-quarter scatter indices
        lg = res.tile([P, TCH, E], bf16)           # all-expert logits per token

        for _rep in range(repeat):
            # ---------- phase A: sim matmul + routing ----------
            for c in range(TCH):
                hsf_t = streams.tile([P, HK, P], f32, tag="hsf_t")
                nc.sync.dma_start(hsf_t[:], hsTf_r[:, :, c * P:(c + 1) * P])
                ps = psum.tile([P, 512], f32, tag="ps")
                for kk in range(HK):
                    nc.tensor.matmul(ps[:], hsf_t[:, kk, :], wk_sb[:, kk, :],
                                     start=(kk == 0), stop=(kk == HK - 1))
                sim = rpool.tile([P, 512], f32, tag="sim")
                nc.vector.tensor_copy(sim[:], ps[:])

                e8i = rpool.tile([P, HEADS, 8], i32, tag="e8i")
                for h in range(HEADS):
                    simx = sim[:, h * NK:(h + 1) * NK]
                    simy = sim[:, 256 + h * NK:256 + (h + 1) * NK]
                    sx = rpool.tile([P, 8], f32, tag="sx")
                    sy = rpool.tile([P, 8], f32, tag="sy")
                    ix = rpool.tile([P, 8], u32, tag="ix")
                    iy = rpool.tile([P, 8], u32, tag="iy")
                    nc.vector.max(sx[:], simx)
                    nc.vector.max_index(ix[:], sx[:], simx)
                    nc.vector.max(sy[:], simy)
                    nc.vector.max_index(iy[:], sy[:], simy)

                    cc = rpool.tile([P, 8, 8], f32, tag="cc")
                    nc.vector.tensor_tensor(cc[:], sx[:, :, None].to_broadcast([P, 8, 8]),
                                            sy[:, None, :].to_broadcast([P, 8, 8]), OP.add)
                    cflat = cc[:].rearrange("p a b -> p (a b)")
                    s8 = rpool.tile([P, 8], f32, tag="s8")
                    pk = rpool.tile([P, 8], u32, tag="pk")
                    nc.vector.max(s8[:], cflat)
                    nc.vector.max_index(pk[:], s8[:], cflat)

                    # softmax over the 8 selected scores
                    d8 = rpool.tile([P, 8], f32, tag="d8")
                    nc.vector.tensor_scalar(d8[:], s8[:], s8[:, 0:1], None, op0=OP.subtract)
                    ex8 = rpool.tile([P, 8], f32, tag="ex8")
                    nc.scalar.activation(ex8[:], d8[:], AF.Exp)
                    z = rpool.tile([P, 1], f32, tag="z")
                    nc.vector.tensor_reduce(z[:], ex8[:], axis=AX.X, op=OP.add)
                    rz = rpool.tile([P, 1], f32, tag="rz")
                    nc.vector.reciprocal(rz[:], z[:])
                    nc.vector.tensor_scalar(p8bf[:, c, h, :], ex8[:], rz[:, 0:1], None,
                                            op0=OP.mult)

                    # expert ids: e8 = ix[pk>>3]*64 + iy[pk&7]
                    pkhu = rpool.tile([P, 8], u32, tag="pkhu")
                    pklu = rpool.tile([P, 8], u32, tag="pklu")
                    nc.vector.tensor_scalar(pkhu[:], pk[:], 3, None, op0=OP.logical_shift_right)
                    nc.vector.tensor_scalar(pklu[:], pk[:], 7, None, op0=OP.bitwise_and)
                    pkh = rpool.tile([P, 8], i32, tag="pkh")
                    pkl = rpool.tile([P, 8], i32, tag="pkl")
                    nc.vector.tensor_copy(pkh[:], pkhu[:])
                    nc.vector.tensor_copy(pkl[:], pklu[:])
                    ixi = rpool.tile([P, 8], i32, tag="ixi")
                    iyi = rpool.tile([P, 8], i32, tag="iyi")
                    nc.vector.tensor_copy(ixi[:], ix[:])
                    nc.vector.tensor_copy(iyi[:], iy[:])

                    ohx = rpool.tile([P, 8, 8], i32, tag="ohx")
                    ohy = rpool.tile([P, 8, 8], i32, tag="ohy")
                    nc.vector.tensor_tensor(ohx[:], pkh[:, :, None].to_broadcast([P, 8, 8]),
                                            iota8[:, None, :].to_broadcast([P, 8, 8]), OP.is_equal)
                    nc.vector.tensor_tensor(ohy[:], pkl[:, :, None].to_broadcast([P, 8, 8]),
                                            iota8[:, None, :].to_broadcast([P, 8, 8]), OP.is_equal)
                    mx = rpool.tile([P, 8, 8], i32, tag="mx")
                    my = rpool.tile([P, 8, 8], i32, tag="my")
                    nc.vector.tensor_tensor(mx[:], ohx[:],
                                            ixi[:, None, :].to_broadcast([P, 8, 8]), OP.mult)
                    nc.vector.tensor_tensor(my[:], ohy[:],
                                            iyi[:, None, :].to_broadcast([P, 8, 8]), OP.mult)
                    ixs = rpool.tile([P, 8], i32, tag="ixs")
                    iys = rpool.tile([P, 8], i32, tag="iys")
                    with nc.allow_low_precision(reason="int32 onehot-select, exact"):
                        nc.vector.tensor_reduce(ixs[:], mx[:], axis=AX.X, op=OP.add)
                        nc.vector.tensor_reduce(iys[:], my[:], axis=AX.X, op=OP.add)
                    nc.vector.scalar_tensor_tensor(e8i[:, h, :], ixs[:], NK, iys[:],
                                                   op0=OP.mult, op1=OP.add)

                # quarter-local scatter indices, wrong-quarter -> negative int16
                for q in range(4):
                    t1 = rpool.tile([P, HEADS, 8], i32, tag="t1")
                    nc.vector.tensor_scalar(t1[:], e8i[:], QE * q + QE, None,
                                            op0=OP.subtract)
                    t2 = rpool.tile([P, HEADS, 8], i32, tag="t2")
                    nc.vector.tensor_scalar(t2[:], t1[:], 4095, None,
                                            op0=OP.bitwise_and)
                    nc.vector.tensor_scalar(idx16[:, c, q, :, :], t2[:], 3072, None,
                                            op0=OP.subtract)

            nc.sync.dma_start(hsTb_sb[:], hsT_b[:].rearrange("(o p) n -> p o n", p=P))

            # ---------- phase B: all-expert logits Lg (bf16) ----------
            # chunk-pair outer order so lg for early token chunks is complete at
            # mid-B, letting the scatter->mul->silu chain start early (de_T is
            # streamed twice; +8MB DMA, well under the PE span).
            for cp in range(TCH // 2):
                for ec in range(E // 256):
                    de_t = streams.tile([P, HK, 256], bf16, tag="de_t")
                    nc.sync.dma_start(de_t[:], deT_r[:, :, ec * 256:(ec + 1) * 256])
                    for c in (2 * cp, 2 * cp + 1):
                        ps = psum.tile([P, 256], f32, tag="ps")
                        for kk in range(HK):
                            nc.tensor.matmul(ps[:], hsTb_sb[:, kk, c * P:(c + 1) * P],
                                             de_t[:, kk, :], start=(kk == 0), stop=(kk == HK - 1))
                        nc.scalar.activation(lg[:, c, ec * 256:(ec + 1) * 256], ps[:], AF.Copy)

            # ---------- phase C: dense up-proj + silu ----------
            for ic4 in range(ICH // 2):
                wup_t = streams.tile([P, HK, 2 * P], bf16, tag="wup_t")
                nc.sync.dma_start(wup_t[:], wupT_r[:, :, ic4 * 2 * P:(ic4 + 1) * 2 * P])
                for j in range(2):
                    ic = ic4 * 2 + j
                    ps = psum.tile([P, 512], f32, tag="ps")
                    for kk in range(HK):
                        nc.tensor.matmul(ps[:], wup_t[:, kk, j * P:(j + 1) * P],
                                         hsTb_sb[:, kk, :],
                                         start=(kk == 0), stop=(kk == HK - 1))
                    nc.scalar.activation(y1T[:, ic, :], ps[:], AF.Silu)

            # ---------- phase A5/A6: scatter + dense silu-weight build ----------
            for c in range(TCH):
                sc = scpool.tile([P, E], bf16, tag="sc")
                for h in range(HEADS):
                    ph = rpool.tile([P, E], bf16, tag="ph")
                    for q in range(4):
                        nc.gpsimd.local_scatter(
                            ph[:, q * QE:(q + 1) * QE], p8bf[:, c, h, :],
                            idx16[:, c, q, h, :], channels=P, num_elems=QE, num_idxs=8)
                    tt = rpool.tile([P, E], bf16, tag="ph")
                    nc.vector.tensor_tensor(tt[:], lg[:, c, :], ph[:], OP.mult)
                    if h == 0:
                        nc.scalar.activation(sc[:], tt[:], AF.Silu)
                    else:
                        wh = scpool.tile([P, E], bf16, tag="wh")
                        nc.scalar.activation(wh[:], tt[:], AF.Silu)
                        nc.vector.tensor_tensor(sc[:], sc[:], wh[:], OP.add)
                # S.T for this token chunk in one xbar transpose (SBUF->SBUF):
                # row e=o*128+p of sc.T lands at sT[p, o, c-slice]
                nc.scalar.dma_start_transpose(sT[:, :, c * P:(c + 1) * P], sc[:])

            # ---------- phase D: down-proj + expert combine, fused in PSUM ----------
            # All 8 output d-chunks accumulate concurrently (8 PSUM banks) so each
            # weight row-block is loaded once, in one large DMA.
            ps_d = [psum.tile([P, 512], f32, tag="ps", name=f"ps_d{dc}")
                    for dc in range(DCH)]
            for ic in range(ICH):
                wd_t = wstream.tile([P, H], bf16, tag="wd_t")
                nc.sync.dma_start(wd_t[:], wdownT_r[:, ic, :])
                for dc in range(DCH):
                    nc.tensor.matmul(ps_d[dc][:], wd_t[:, dc * P:(dc + 1) * P],
                                     y1T[:, ic, :], start=(ic == 0), stop=False)
            for ic in range(ICH):
                ue_t = wstream.tile([P, H], bf16, tag="ue_t")
                nc.sync.dma_start(ue_t[:], ue_r[:, ic, :])
                for dc in range(DCH):
                    nc.tensor.matmul(ps_d[dc][:], ue_t[:, dc * P:(dc + 1) * P],
                                     sT[:, ic, :], start=False, stop=(ic == ICH - 1))
            for dc in range(DCH):
                ot = outp.tile([P, 512], f32, tag="ot")
                nc.vector.tensor_copy(ot[:], ps_d[dc][:])
                nc.gpsimd.dma_start(outT_d[dc * P:(dc + 1) * P, :], ot[:])


    nc.compile()
    return nc


def _host_prep(hidden_states, W_up, W_down, W_q, keys, down_embed, up_embed):
    bf = ml_dtypes.bfloat16
    hs = np.asarray(hidden_states, dtype=np.float32).reshape(B * T, H)
    W_up = np.asarray(W_up, dtype=np.float32)
    W_down = np.asarray(W_down, dtype=np.float32)
    W_q = np.asarray(W_q, dtype=np.float32)
    keys = np.asarray(keys, dtype=np.float32)
    down_embed = np.asarray(down_embed, dtype=np.float32)
    up_embed = np.asarray(up_embed, dtype=np.float32)

    # compose product-key similarity: WK[(p2,h,k), d] = sum_r Wq[(p2,h,r), d]*keys[h,k,p2,r]
    Wq3 = W_q.reshape(2, HEADS, NK, H).astype(np.float64)
    WK = np.einsum("phrd,hkpr->phkd", Wq3, keys.astype(np.float64))
    WK_T = np.ascontiguousarray(WK.reshape(512, H).T.astype(np.float32))  # [H, 512]

    shared = {
        "wk": WK_T,
        "wupT": np.ascontiguousarray(W_up.T).astype(bf),            # [H, I]
        "wdownT": np.ascontiguousarray(W_down.T).astype(bf),        # [I, H]
        "deT": np.ascontiguousarray(down_embed.T).astype(bf),       # [H, E]
        "ue": np.ascontiguousarray(up_embed).astype(bf),            # [E, H]
    }
    in_maps = []
    for i in range(NCORES):
        shard = hs[i * NT:(i + 1) * NT]                              # [NT, H]
        hsT = np.ascontiguousarray(shard.T)                          # [H, NT]
        m = dict(shared)
        m["hsT_f"] = hsT
        m["hsT_b"] = hsT.astype(bf)
        in_maps.append(m)
    return in_maps


def kernel(hidden_states, W_up, W_down, W_q, keys, down_embed, up_embed,
           trace=False):
    from concourse.bass_utils import run_bass_kernel_spmd

    if "nc" not in _CACHE:
        _CACHE["nc"] = _build_program()
    nc = _CACHE["nc"]

    in_maps = _host_prep(hidden_states, W_up, W_down, W_q, keys,
                         down_embed, up_embed)
    res = run_bass_kernel_spmd(nc, in_maps, list(range(NCORES)), trace=trace)
    out = np.empty((B * T, H), np.float32)
    for i, r in enumerate(res.results):
        out[i * NT:(i + 1) * NT] = r["outT"].T
    if trace:
        kernel.last_results = res
    return out.reshape(B, T, H)

